# revision 6
# baseline (speedup 1.0000x reference)
"""ContextualNeuronPool Trainium2 kernel (8-core SPMD).

Math (per token t, with K=8 selected pool entries p_k = idx[t,k], w = softmax(pattern_weights[t])):
    combined[t, f] = sum_k w_k * bp_eff[p_k, f]                  (base term, via routing matrix A)
                   + (sum_k w_k * (G[p_k] @ x[t])) @ adj_proj    (modulation term, via MoE grouping)
    out[t] = gelu(combined[t]) @ W2^T + w2_b
where G[p] = cm_w block [64, 1024] for pool entry p and bp_eff folds the cm_b bias:
    bp_eff = base_patterns + cm_b.reshape(P, M) @ adj_proj       (exact constant folding, host side)

Sharding:
  Phase A (expert-sharded): core c owns pool entries [64c, 64c+64). Host groups (token, k)
  pairs by pool entry (integer routing logistics only), pads each entry's token list to 128,
  and ships the gathered x columns (bf16) so each entry's modulation vectors come from one
  [128d x 128pair] x [128d x 64m] matmul chain. Pair vectors go to a DRAM table, AllGather'd.
  Phase B (token-sharded): core c owns tokens [512c, 512c+512). Softmax, routing matrix A,
  base-term matmul A @ bp_eff, + modulation @ adj_proj, gelu, @ W2^T -- all dense matmuls.
"""

import numpy as np
import ml_dtypes

import concourse.bacc as bacc
import concourse.bass as bass
import concourse.tile as tile
import concourse.mybir as mybir
from concourse.bass_utils import run_bass_kernel_spmd
from concourse.masks import make_identity

BF16 = mybir.dt.bfloat16
F32 = mybir.dt.float32
I32 = mybir.dt.int32
AF = mybir.ActivationFunctionType
ALU = mybir.AluOpType

POOL, D, DFF, M = 512, 1024, 4096, 64
B, S, K = 2, 2048, 8
NCORES = 8
NTOK = B * S                  # 4096 tokens
T = NTOK // NCORES            # 512 tokens per core
EPC = POOL // NCORES          # 64 experts (pool entries) per core
PADE = 128                    # padded pairs per expert
NPAIR = EPC * PADE            # 8192 rows in per-core pair table
NAG = NCORES * NPAIR          # 65536 rows in allgathered table
DC = D // 128                 # 8 contraction chunks
TT = T // 128                 # 4 token tiles per core
PC = POOL // 128              # 4 pool chunks
FT = DFF // 128               # 32 d_ff tiles


def _build_program():
    nc = bacc.Bacc("TRN2", target_bir_lowering=False, debug=False, num_devices=NCORES)

    xgt_d = nc.dram_tensor("xgt", [D, NPAIR], BF16, kind="ExternalInput")
    cmt_d = nc.dram_tensor("cmt", [D, EPC * M], BF16, kind="ExternalInput")
    bp_d = nc.dram_tensor("bp", [POOL, DFF], BF16, kind="ExternalInput")
    adj_d = nc.dram_tensor("adjp", [M, DFF], BF16, kind="ExternalInput")
    w2t_d = nc.dram_tensor("w2t", [DFF, D], BF16, kind="ExternalInput")
    idx_d = nc.dram_tensor("idx", [T, K], I32, kind="ExternalInput")
    pw_d = nc.dram_tensor("pw", [T, K], F32, kind="ExternalInput")
    gidx_d = nc.dram_tensor("gidx", [128, TT * K], I32, kind="ExternalInput")
    out_d = nc.dram_tensor("out", [T, D], F32, kind="ExternalOutput")

    with tile.TileContext(nc) as tc:
        with tc.tile_pool(name="const", bufs=1) as const, \
             tc.tile_pool(name="xg", bufs=2) as xg_pool, \
             tc.tile_pool(name="cm", bufs=2) as cm_pool, \
             tc.tile_pool(name="pra", bufs=4) as pr_pool, \
             tc.tile_pool(name="small", bufs=1) as small, \
             tc.tile_pool(name="abuild", bufs=2) as ab_pool, \
             tc.tile_pool(name="rg", bufs=2) as rg_pool, \
             tc.tile_pool(name="w2s", bufs=3) as w2_pool, \
             tc.tile_pool(name="outp", bufs=3) as out_pool, \
             tc.tile_pool(name="dram", bufs=1, space="DRAM") as dram:

            # ---------------- constants / small inputs ----------------
            ident = const.tile([128, 128], BF16)
            make_identity(nc, ident[:])
            iota_i = const.tile([128, POOL], I32)
            nc.gpsimd.iota(iota_i[:], pattern=[[1, POOL]], base=0, channel_multiplier=0)
            iota_f = const.tile([128, POOL], F32)
            nc.vector.tensor_copy(out=iota_f[:], in_=iota_i[:])

            idxf = small.tile([128, TT, K], F32)
            w_sb = small.tile([128, TT, K], F32)
            gidx_sb = small.tile([128, TT * K], I32)
            nc.sync.dma_start(out=gidx_sb[:], in_=gidx_d[:, :])

            idx_i = small.tile([128, TT, K], I32)
            pw_sb = small.tile([128, TT, K], F32)
            for ti in range(TT):
                nc.sync.dma_start(out=idx_i[:, ti], in_=idx_d[ti * 128:(ti + 1) * 128, :])
                nc.sync.dma_start(out=pw_sb[:, ti], in_=pw_d[ti * 128:(ti + 1) * 128, :])

            # softmax over k (per token) + int->float cast of indices
            negmax = small.tile([128, TT, 1], F32)
            sume = small.tile([128, TT, 1], F32)
            rec = small.tile([128, TT, 1], F32)
            for ti in range(TT):
                nc.vector.reduce_max(out=negmax[:, ti], in_=pw_sb[:, ti],
                                     axis=mybir.AxisListType.X, negate=True)
                nc.scalar.activation(out=w_sb[:, ti], in_=pw_sb[:, ti], func=AF.Exp,
                                     bias=negmax[:, ti], scale=1.0, accum_out=sume[:, ti])
                nc.vector.reciprocal(out=rec[:, ti], in_=sume[:, ti])
                nc.vector.tensor_scalar_mul(out=w_sb[:, ti], in0=w_sb[:, ti], scalar1=rec[:, ti])
                nc.vector.tensor_copy(out=idxf[:, ti], in_=idx_i[:, ti])

            pair_tab = dram.tile([NPAIR, M], BF16)
            ag_tab = dram.tile([NAG, M], BF16, addr_space="Shared")

            at_tiles = []  # A^T chunk tiles [128 pool, T] bf16
            for pj in range(PC):
                at_tiles.append(const.tile([128, T], BF16, tag=f"at{pj}", name=f"at{pj}"))
            wmodT = const.tile([M, T], BF16, tag="wmodT")

            with tc.tile_pool(name="psA", bufs=4, space="PSUM") as psA, \
                 tc.tile_pool(name="psT", bufs=2, space="PSUM") as psT:
                # ------------ phase A: per-expert modulation vectors ------------
                GRP = 8  # experts per load group
                for g in range(EPC // GRP):
                    xg = xg_pool.tile([128, DC, GRP * PADE], BF16)
                    cm = cm_pool.tile([128, DC, GRP * M], BF16)
                    for j in range(DC):
                        nc.sync.dma_start(
                            out=xg[:, j], in_=xgt_d[j * 128:(j + 1) * 128,
                                                    g * GRP * PADE:(g + 1) * GRP * PADE])
                        nc.sync.dma_start(
                            out=cm[:, j], in_=cmt_d[j * 128:(j + 1) * 128,
                                                    g * GRP * M:(g + 1) * GRP * M])
                    for s in range(GRP):
                        e = g * GRP + s
                        ps = psA.tile([128, M], F32)
                        for j in range(DC):
                            nc.tensor.matmul(ps[:],
                                             lhsT=xg[:, j, s * PADE:(s + 1) * PADE],
                                             rhs=cm[:, j, s * M:(s + 1) * M],
                                             start=(j == 0), stop=(j == DC - 1))
                        pr = pr_pool.tile([128, M], BF16)
                        nc.vector.tensor_copy(out=pr[:], in_=ps[:])
                        nc.sync.dma_start(out=pair_tab[e * PADE:(e + 1) * PADE, :], in_=pr[:])

                nc.gpsimd.collective_compute(
                    "AllGather", ALU.bypass,
                    replica_groups=[list(range(NCORES))],
                    ins=[pair_tab[:].opt()], outs=[ag_tab[:].opt()],
                )

                # ------------ routing matrix A (token-major) + transpose ------------
                for ti in range(TT):
                    a_t = ab_pool.tile([128, POOL], BF16)
                    for k in range(K):
                        if k == 0:
                            nc.vector.tensor_scalar(out=a_t[:], in0=iota_f[:],
                                                    scalar1=idxf[:, ti, k:k + 1],
                                                    scalar2=w_sb[:, ti, k:k + 1],
                                                    op0=ALU.is_equal, op1=ALU.mult)
                        else:
                            tmp = ab_pool.tile([128, POOL], BF16, tag="atmp")
                            nc.vector.tensor_scalar(out=tmp[:], in0=iota_f[:],
                                                    scalar1=idxf[:, ti, k:k + 1],
                                                    scalar2=w_sb[:, ti, k:k + 1],
                                                    op0=ALU.is_equal, op1=ALU.mult)
                            nc.vector.tensor_tensor(out=a_t[:], in0=a_t[:], in1=tmp[:], op=ALU.add)
                    for pj in range(PC):
                        pst = psT.tile([128, 128], BF16)
                        nc.tensor.transpose(pst[:], a_t[:, pj * 128:(pj + 1) * 128], ident[:])
                        nc.vector.tensor_copy(out=at_tiles[pj][:, ti * 128:(ti + 1) * 128],
                                              in_=pst[:])

                # ------------ gather pair vectors, weighted k-sum -> wmodT ------------
                for ti in range(TT):
                    rg = rg_pool.tile([128, K, M], BF16)
                    for k in range(K):
                        nc.gpsimd.indirect_dma_start(
                            out=rg[:, k], out_offset=None,
                            in_=ag_tab[:],
                            in_offset=bass.IndirectOffsetOnAxis(
                                ap=gidx_sb[:, ti * K + k: ti * K + k + 1], axis=0),
                        )
                    rw = rg_pool.tile([128, K, M], F32, tag="rw")
                    for k in range(K):
                        nc.vector.tensor_scalar_mul(out=rw[:, k], in0=rg[:, k],
                                                    scalar1=w_sb[:, ti, k:k + 1])
                    wmod = rg_pool.tile([128, M], F32, tag="wmod")
                    nc.vector.reduce_sum(out=wmod[:], in_=rw[:].rearrange("p k m -> p m k"),
                                         axis=mybir.AxisListType.X)
                    wmod_bf = rg_pool.tile([128, M], BF16, tag="wmodbf")
                    nc.vector.tensor_copy(out=wmod_bf[:], in_=wmod[:])
                    psw = psT.tile([M, 128], BF16, tag="psw")
                    nc.tensor.transpose(psw[:], wmod_bf[:], ident[:])
                    nc.vector.tensor_copy(out=wmodT[:, ti * 128:(ti + 1) * 128], in_=psw[:])

            # ---------------- phase B1: combined^T = A@bp + wmod@adj, gelu ----------------
            bp_tiles = []
            for pj in range(PC):
                t_ = const.tile([128, DFF], BF16, tag=f"bp{pj}", name=f"bp{pj}")
                nc.sync.dma_start(out=t_[:], in_=bp_d[pj * 128:(pj + 1) * 128, :])
                bp_tiles.append(t_)
            adj_sb = const.tile([M, DFF], BF16, tag="adj")
            nc.sync.dma_start(out=adj_sb[:], in_=adj_d[:, :])

            act_tiles = []
            with tc.tile_pool(name="psB", bufs=4, space="PSUM") as psB:
                for ft in range(FT):
                    psb = psB.tile([128, T], F32)
                    for pj in range(PC):
                        nc.tensor.matmul(psb[:], lhsT=bp_tiles[pj][:, ft * 128:(ft + 1) * 128],
                                         rhs=at_tiles[pj][:], start=(pj == 0), stop=False)
                    nc.tensor.matmul(psb[:], lhsT=adj_sb[:, ft * 128:(ft + 1) * 128],
                                     rhs=wmodT[:], start=False, stop=True)
                    act = const.tile([128, T], BF16, tag=f"act{ft}", name=f"act{ft}")
                    nc.scalar.activation(out=act[:], in_=psb[:], func=AF.Gelu)
                    act_tiles.append(act)

            # ---------------- phase B2: out = act^T @ W2^T ----------------
            with tc.tile_pool(name="psO", bufs=1, space="PSUM") as psO:
                out_ps = {}
                for t in range(TT):
                    for dd in range(2):
                        out_ps[(t, dd)] = psO.tile([128, 512], F32, tag=f"o{t}_{dd}",
                                                   name=f"ops{t}_{dd}")
                for fc in range(FT):
                    w2c = w2_pool.tile([128, D], BF16)
                    nc.sync.dma_start(out=w2c[:], in_=w2t_d[fc * 128:(fc + 1) * 128, :])
                    for t in range(TT):
                        for dd in range(2):
                            nc.tensor.matmul(out_ps[(t, dd)][:],
                                             lhsT=act_tiles[fc][:, t * 128:(t + 1) * 128],
                                             rhs=w2c[:, dd * 512:(dd + 1) * 512],
                                             start=(fc == 0), stop=(fc == FT - 1))
                for t in range(TT):
                    for dd in range(2):
                        ob = out_pool.tile([128, 512], F32)
                        nc.vector.tensor_copy(out=ob[:], in_=out_ps[(t, dd)][:])
                        nc.sync.dma_start(
                            out=out_d[t * 128:(t + 1) * 128, dd * 512:(dd + 1) * 512],
                            in_=ob[:])

    nc.compile()
    return nc


def _prepare_inputs(x, selected_indices, pattern_weights, base_patterns, cm_w, cm_b,
                    adj_proj, w2_w):
    bf = ml_dtypes.bfloat16
    x2 = np.ascontiguousarray(x.reshape(NTOK, D), dtype=np.float32)
    idx = np.ascontiguousarray(selected_indices.reshape(NTOK, K)).astype(np.int32)
    pw = np.ascontiguousarray(pattern_weights.reshape(NTOK, K), dtype=np.float32)

    # exact constant folding of the cm_b bias into the base patterns
    bp_eff = base_patterns.astype(np.float32) + cm_b.reshape(POOL, M).astype(np.float32) @ adj_proj.astype(np.float32)
    bp_bf = bp_eff.astype(bf)
    adj_bf = adj_proj.astype(bf)
    w2t_bf = np.ascontiguousarray(w2_w.T).astype(bf)
    x2t_bf = np.ascontiguousarray(x2.T).astype(bf)  # [D, NTOK]

    # group (t, k) pairs by pool entry
    flat_e = idx.ravel()                       # expert of pair j = t*K + k
    order = np.argsort(flat_e, kind="stable")  # pairs sorted by (expert, t, k)
    counts = np.bincount(flat_e, minlength=POOL)
    assert counts.max() <= PADE, f"expert overflow: max {counts.max()} > {PADE}"
    starts = np.zeros(POOL, dtype=np.int64)
    starts[1:] = np.cumsum(counts)[:-1]
    # rank of each sorted pair within its expert segment
    ranks_sorted = np.arange(NTOK * K, dtype=np.int64) - starts[flat_e[order]]
    # allgather row of each pair (in original (t, k) layout)
    e_sorted = flat_e[order]
    agrow_sorted = (e_sorted // EPC) * NPAIR + (e_sorted % EPC) * PADE + ranks_sorted
    agrow = np.empty(NTOK * K, dtype=np.int64)
    agrow[order] = agrow_sorted
    agrow = agrow.reshape(NTOK, K)

    tok_sorted = (np.arange(NTOK * K, dtype=np.int64) // K)[order]

    in_maps = []
    for c in range(NCORES):
        xgt = np.zeros((D, NPAIR), dtype=bf)
        for s in range(EPC):
            e = c * EPC + s
            seg = tok_sorted[starts[e]:starts[e] + counts[e]]
            xgt[:, s * PADE:s * PADE + len(seg)] = x2t_bf[:, seg]
        cmt = np.ascontiguousarray(
            cm_w.reshape(POOL, M, D)[c * EPC:(c + 1) * EPC].transpose(2, 0, 1).reshape(D, EPC * M)
        ).astype(bf)
        agrow_loc = agrow[c * T:(c + 1) * T]            # [T, K]
        gidx = np.ascontiguousarray(
            agrow_loc.reshape(TT, 128, K).transpose(1, 0, 2).reshape(128, TT * K)
        ).astype(np.int32)
        in_maps.append({
            "xgt": xgt,
            "cmt": cmt,
            "bp": bp_bf,
            "adjp": adj_bf,
            "w2t": w2t_bf,
            "idx": np.ascontiguousarray(idx[c * T:(c + 1) * T]),
            "pw": np.ascontiguousarray(pw[c * T:(c + 1) * T]),
            "gidx": gidx,
        })
    return in_maps


def _run(inputs, trace=False):
    nc = _build_program()
    in_maps = _prepare_inputs(
        inputs["x"], inputs["selected_indices"], inputs["pattern_weights"],
        inputs["base_patterns"], inputs["cm_w"], inputs["cm_b"],
        inputs["adj_proj"], inputs["w2_w"])
    res = run_bass_kernel_spmd(nc, in_maps, core_ids=list(range(NCORES)), trace=trace)
    out = np.concatenate([res.results[c]["out"] for c in range(NCORES)], axis=0)
    out = out + np.asarray(inputs["w2_b"], dtype=np.float32)[None, :]
    return out.reshape(B, S, D).astype(np.float32), res


def kernel(**inputs) -> np.ndarray:
    out, _ = _run(inputs, trace=False)
    return out


# revision 10
# speedup vs baseline: 1161.8006x; 1161.8006x over previous
"""ContextualNeuronPool Trainium2 kernel (8-core SPMD).

Math (per token t, with K=8 selected pool entries p_k = idx[t,k], w = softmax(pattern_weights[t])):
    combined[t, f] = sum_k w_k * bp_eff[p_k, f]                  (base term, via routing matrix A)
                   + (sum_k w_k * (G[p_k] @ x[t])) @ adj_proj    (modulation term, via MoE grouping)
    out[t] = gelu(combined[t]) @ W2^T + w2_b
where G[p] = cm_w block [64, 1024] for pool entry p and bp_eff folds the cm_b bias:
    bp_eff = base_patterns + cm_b.reshape(P, M) @ adj_proj       (exact constant folding, host side)

Sharding:
  Phase A (expert-sharded): core c owns pool entries [64c, 64c+64). Host groups (token, k)
  pairs by pool entry (integer routing logistics only). Per core, entries are sorted by
  token count into 64 slots; slot s is padded to a global slot_sizes[s] (max over cores,
  16-aligned) so all cores run an identical program. Host ships the gathered x columns
  (bf16); each slot's modulation vectors come from one [128d x m] x [128d x 64] matmul
  chain. Pair vectors go to a DRAM table, AllGather'd across the 8 cores.
  Phase B (token-sharded): core c owns tokens [512c, 512c+512). Softmax, routing matrix A,
  base-term matmul A @ bp_eff (overlaps the AllGather), + modulation @ adj_proj, gelu,
  @ W2^T -- all dense matmuls.
"""

import numpy as np
import ml_dtypes

import concourse.bacc as bacc
import concourse.bass as bass
import concourse.tile as tile
import concourse.mybir as mybir
from concourse.bass_utils import run_bass_kernel_spmd
from concourse.masks import make_identity

BF16 = mybir.dt.bfloat16
F32 = mybir.dt.float32
I32 = mybir.dt.int32
AF = mybir.ActivationFunctionType
ALU = mybir.AluOpType

POOL, D, DFF, M = 512, 1024, 4096, 64
B, S, K = 2, 2048, 8
NCORES = 8
NTOK = B * S                  # 4096 tokens
T = NTOK // NCORES            # 512 tokens per core
EPC = POOL // NCORES          # 64 experts (pool entries) per core
DC = D // 128                 # 8 contraction chunks
TT = T // 128                 # 4 token tiles per core
PC = POOL // 128              # 4 pool chunks
FT = DFF // 128               # 32 d_ff tiles
GRP = 16                      # expert slots per DMA load group


def _build_program(slot_sizes):
    slot_off = np.concatenate([[0], np.cumsum(slot_sizes)]).astype(int)
    TW = int(slot_off[-1])          # total packed pair-table width
    NAG = NCORES * TW

    nc = bacc.Bacc("TRN2", target_bir_lowering=False, debug=False, num_devices=NCORES)

    xgt_d = nc.dram_tensor("xgt", [D, TW], BF16, kind="ExternalInput")
    cmt_d = nc.dram_tensor("cmt", [D, EPC * M], BF16, kind="ExternalInput")
    bp_d = nc.dram_tensor("bp", [POOL, DFF], BF16, kind="ExternalInput")
    adj_d = nc.dram_tensor("adjp", [M, DFF], BF16, kind="ExternalInput")
    w2t_d = nc.dram_tensor("w2t", [DFF, D], BF16, kind="ExternalInput")
    idx_d = nc.dram_tensor("idx", [T, K], I32, kind="ExternalInput")
    pw_d = nc.dram_tensor("pw", [T, K], F32, kind="ExternalInput")
    gidx_d = nc.dram_tensor("gidx", [128, TT * K], I32, kind="ExternalInput")
    out_d = nc.dram_tensor("out", [T, D], F32, kind="ExternalOutput")

    with tile.TileContext(nc) as tc:
        with tc.tile_pool(name="const", bufs=1) as const, \
             tc.tile_pool(name="xg", bufs=2) as xg_pool, \
             tc.tile_pool(name="cm", bufs=2) as cm_pool, \
             tc.tile_pool(name="pra", bufs=4) as pr_pool, \
             tc.tile_pool(name="small", bufs=1) as small, \
             tc.tile_pool(name="abuild", bufs=2) as ab_pool, \
             tc.tile_pool(name="rg", bufs=8) as rg_pool, \
             tc.tile_pool(name="rw", bufs=2) as rw_pool, \
             tc.tile_pool(name="w2s", bufs=2) as w2_pool, \
             tc.tile_pool(name="outp", bufs=2) as out_pool, \
             tc.tile_pool(name="dram", bufs=1, space="DRAM") as dram:

            # ---------------- constants / small inputs ----------------
            ident = const.tile([128, 128], BF16)
            make_identity(nc, ident[:])
            iota_f = const.tile([128, POOL], F32)
            nc.gpsimd.iota(iota_f[:], pattern=[[1, POOL]], base=0, channel_multiplier=0,
                           allow_small_or_imprecise_dtypes=True)

            idxf = small.tile([128, TT, K], F32)
            w_sb = small.tile([128, TT, K], F32)
            gidx_sb = small.tile([128, TT * K], I32)
            nc.sync.dma_start(out=gidx_sb[:], in_=gidx_d[:, :])

            idx_i = small.tile([128, TT, K], I32)
            pw_sb = small.tile([128, TT, K], F32)
            for ti in range(TT):
                nc.sync.dma_start(out=idx_i[:, ti], in_=idx_d[ti * 128:(ti + 1) * 128, :])
                nc.sync.dma_start(out=pw_sb[:, ti], in_=pw_d[ti * 128:(ti + 1) * 128, :])

            # softmax over k (per token) + int->float cast of indices
            negmax = small.tile([128, TT, 1], F32)
            sume = small.tile([128, TT, 1], F32)
            rec = small.tile([128, TT, 1], F32)
            for ti in range(TT):
                nc.vector.reduce_max(out=negmax[:, ti], in_=pw_sb[:, ti],
                                     axis=mybir.AxisListType.X, negate=True)
                nc.scalar.activation(out=w_sb[:, ti], in_=pw_sb[:, ti], func=AF.Exp,
                                     bias=negmax[:, ti], scale=1.0, accum_out=sume[:, ti])
                nc.vector.reciprocal(out=rec[:, ti], in_=sume[:, ti])
                nc.vector.tensor_scalar_mul(out=w_sb[:, ti], in0=w_sb[:, ti], scalar1=rec[:, ti])
                nc.vector.tensor_copy(out=idxf[:, ti], in_=idx_i[:, ti])

            pair_tab = dram.tile([TW, M], BF16)
            ag_tab = dram.tile([NAG, M], BF16, addr_space="Shared")

            at_tiles = []  # A^T chunk tiles [128 pool, T] bf16
            for pj in range(PC):
                at_tiles.append(const.tile([128, T], BF16, tag=f"at{pj}", name=f"at{pj}"))
            wmodT = const.tile([M, T], BF16, tag="wmodT")
            stage_tiles = []  # staged base term combined^T tiles [128 f, T] bf16
            act_tiles = []
            for ft in range(FT):
                stage_tiles.append(const.tile([128, T], BF16, tag=f"stg{ft}", name=f"stg{ft}"))
                act_tiles.append(const.tile([128, T], BF16, tag=f"act{ft}", name=f"act{ft}"))

            with tc.tile_pool(name="psA", bufs=3, space="PSUM") as psA, \
                 tc.tile_pool(name="psT", bufs=2, space="PSUM") as psT, \
                 tc.tile_pool(name="psB", bufs=2, space="PSUM") as psB, \
                 tc.tile_pool(name="bpp", bufs=1) as bp_pool:
                # ------------ phase A: per-slot modulation vectors ------------
                for g in range(EPC // GRP):
                    glo, ghi = slot_off[g * GRP], slot_off[(g + 1) * GRP]
                    gw = int(ghi - glo)
                    xg = [xg_pool.tile([128, gw], BF16, tag=f"xgc{j}", name=f"xg{g}_{j}", bufs=2)
                          for j in range(DC)]
                    cm = [cm_pool.tile([128, GRP * M], BF16, tag=f"cmc{j}", name=f"cm{g}_{j}", bufs=2)
                          for j in range(DC)]
                    for j in range(DC):
                        nc.sync.dma_start(out=xg[j][:], in_=xgt_d[j * 128:(j + 1) * 128, glo:ghi])
                        nc.sync.dma_start(
                            out=cm[j][:], in_=cmt_d[j * 128:(j + 1) * 128,
                                                    g * GRP * M:(g + 1) * GRP * M])
                    for s in range(GRP):
                        sl = g * GRP + s
                        m_s = int(slot_sizes[sl])
                        lo = int(slot_off[sl] - glo)
                        ps = psA.tile([128, M], F32)
                        for j in range(DC):
                            nc.tensor.matmul(ps[:m_s, :],
                                             lhsT=xg[j][:, lo:lo + m_s],
                                             rhs=cm[j][:, s * M:(s + 1) * M],
                                             start=(j == 0), stop=(j == DC - 1))
                        pr = pr_pool.tile([128, M], BF16)
                        nc.vector.tensor_copy(out=pr[:m_s, :], in_=ps[:m_s, :])
                        nc.gpsimd.dma_start(
                            out=pair_tab[int(slot_off[sl]):int(slot_off[sl]) + m_s, :],
                            in_=pr[:m_s, :])

                nc.gpsimd.collective_compute(
                    "AllGather", ALU.bypass,
                    replica_groups=[list(range(NCORES))],
                    ins=[pair_tab[:].opt()], outs=[ag_tab[:].opt()],
                )

                # gather pair vectors (issued immediately after the collective)
                rg_tiles = {}
                for ti in range(TT):
                    for k in range(K):
                        rgt = rg_pool.tile([128, M], BF16, tag="rg", name=f"rg{ti}_{k}")
                        nc.gpsimd.indirect_dma_start(
                            out=rgt[:], out_offset=None,
                            in_=ag_tab[:],
                            in_offset=bass.IndirectOffsetOnAxis(
                                ap=gidx_sb[:, ti * K + k: ti * K + k + 1], axis=0),
                        )
                        rg_tiles[(ti, k)] = rgt

                # ------------ routing matrix A (token-major) + transpose ------------
                for ti in range(TT):
                    a_t = ab_pool.tile([128, POOL], BF16)
                    for k in range(K):
                        if k == 0:
                            nc.vector.tensor_scalar(out=a_t[:], in0=iota_f[:],
                                                    scalar1=idxf[:, ti, k:k + 1],
                                                    scalar2=w_sb[:, ti, k:k + 1],
                                                    op0=ALU.is_equal, op1=ALU.mult)
                        else:
                            tmp = ab_pool.tile([128, POOL], BF16, tag="atmp")
                            nc.vector.tensor_scalar(out=tmp[:], in0=iota_f[:],
                                                    scalar1=idxf[:, ti, k:k + 1],
                                                    scalar2=w_sb[:, ti, k:k + 1],
                                                    op0=ALU.is_equal, op1=ALU.mult)
                            nc.vector.tensor_tensor(out=a_t[:], in0=a_t[:], in1=tmp[:], op=ALU.add)
                    for pj in range(PC):
                        pst = psT.tile([128, 128], BF16)
                        nc.tensor.transpose(pst[:], a_t[:, pj * 128:(pj + 1) * 128], ident[:])
                        nc.vector.tensor_copy(out=at_tiles[pj][:, ti * 128:(ti + 1) * 128],
                                              in_=pst[:])

                # ------ pass 1: base term combined^T = A @ bp_eff (overlaps AllGather) ------
                bp_tiles = []
                for pj in range(PC):
                    t_ = bp_pool.tile([128, DFF], BF16, tag=f"bp{pj}", name=f"bp{pj}")
                    nc.gpsimd.dma_start(out=t_[:], in_=bp_d[pj * 128:(pj + 1) * 128, :])
                    bp_tiles.append(t_)
                for ft in range(FT):
                    psb = psB.tile([128, T], F32)
                    for pj in range(PC):
                        nc.tensor.matmul(psb[:], lhsT=bp_tiles[pj][:, ft * 128:(ft + 1) * 128],
                                         rhs=at_tiles[pj][:], start=(pj == 0), stop=(pj == PC - 1))
                    nc.vector.tensor_copy(out=stage_tiles[ft][:], in_=psb[:])

                # ------------ weighted k-sum of gathered pair vectors -> wmodT ------------
                for ti in range(TT):
                    rw = rw_pool.tile([128, K, M], F32, tag="rw")
                    for k in range(K):
                        nc.vector.tensor_scalar_mul(out=rw[:, k], in0=rg_tiles[(ti, k)][:],
                                                    scalar1=w_sb[:, ti, k:k + 1])
                    wmod = rw_pool.tile([128, M], F32, tag="wmod")
                    nc.vector.reduce_sum(out=wmod[:], in_=rw[:].rearrange("p k m -> p m k"),
                                         axis=mybir.AxisListType.X)
                    wmod_bf = rw_pool.tile([128, M], BF16, tag="wmodbf")
                    nc.vector.tensor_copy(out=wmod_bf[:], in_=wmod[:])
                    psw = psB.tile([M, 128], BF16, tag="psw", bufs=1)
                    nc.tensor.transpose(psw[:], wmod_bf[:], ident[:])
                    nc.vector.tensor_copy(out=wmodT[:, ti * 128:(ti + 1) * 128], in_=psw[:])

            adj_sb = const.tile([M, DFF], BF16, tag="adj")
            nc.sync.dma_start(out=adj_sb[:], in_=adj_d[:, :])

            # ------- pass 2 (adj term + gelu) fused with first half of W2 matmul -------
            def b2_half(psO, trange, reload_tag):
                for fc in range(FT):
                    w2c = w2_pool.tile([128, D], BF16, tag="w2c", name=f"w2c{reload_tag}")
                    nc.sync.dma_start(out=w2c[:], in_=w2t_d[fc * 128:(fc + 1) * 128, :])
                    for t in trange:
                        for dd in range(2):
                            nc.tensor.matmul(psO[(t, dd)][:],
                                             lhsT=act_tiles[fc][:, t * 128:(t + 1) * 128],
                                             rhs=w2c[:, dd * 512:(dd + 1) * 512],
                                             start=(fc == 0), stop=(fc == FT - 1))

            def drain_half(psO, trange):
                for t in trange:
                    for dd in range(2):
                        ob = out_pool.tile([128, 512], F32)
                        nc.vector.tensor_copy(out=ob[:], in_=psO[(t, dd)][:])
                        nc.sync.dma_start(
                            out=out_d[t * 128:(t + 1) * 128, dd * 512:(dd + 1) * 512],
                            in_=ob[:])

            with tc.tile_pool(name="psOa", bufs=1, space="PSUM") as psOa_pool:
                psOa = {}
                for t in (0, 1):
                    for dd in range(2):
                        psOa[(t, dd)] = psOa_pool.tile([128, 512], F32, tag=f"oa{t}_{dd}",
                                                       name=f"opsa{t}_{dd}")
                with tc.tile_pool(name="psC", bufs=2, space="PSUM") as psC:
                    for ft in range(FT):
                        psc = psC.tile([128, T], F32)
                        nc.tensor.matmul(psc[:], lhsT=adj_sb[:, ft * 128:(ft + 1) * 128],
                                         rhs=wmodT[:], start=True, stop=True)
                        comb = ab_pool.tile([128, T], BF16, tag="atmp", name="comb")
                        nc.vector.tensor_tensor(out=comb[:], in0=stage_tiles[ft][:],
                                                in1=psc[:], op=ALU.add)
                        nc.scalar.activation(out=act_tiles[ft][:], in_=comb[:], func=AF.Gelu)

                    b2_half(psOa, (0, 1), "w2a")
                with tc.tile_pool(name="psOb", bufs=1, space="PSUM") as psOb_pool:
                    psOb = {}
                    for t in (2, 3):
                        for dd in range(2):
                            psOb[(t, dd)] = psOb_pool.tile([128, 512], F32, tag=f"ob{t}_{dd}",
                                                           name=f"opsb{t}_{dd}")
                    b2_half(psOb, (2, 3), "w2b")
                    drain_half(psOa, (0, 1))
                    drain_half(psOb, (2, 3))

    nc.compile()
    return nc


def _routing(idx):
    """Group (t, k) pairs by pool entry; build per-core slot packing (sorted by count)."""
    flat_e = idx.ravel()
    order = np.argsort(flat_e, kind="stable")  # pairs sorted by (expert, t, k)
    counts = np.bincount(flat_e, minlength=POOL)
    starts = np.zeros(POOL, dtype=np.int64)
    starts[1:] = np.cumsum(counts)[:-1]
    tok_sorted = (np.arange(NTOK * K, dtype=np.int64) // K)[order]

    # per core: experts sorted by count desc -> slots
    slot_expert = np.zeros((NCORES, EPC), dtype=np.int64)
    for c in range(NCORES):
        cnt = counts[c * EPC:(c + 1) * EPC]
        slot_expert[c] = c * EPC + np.argsort(-cnt, kind="stable")
    slot_counts = counts[slot_expert]                       # [NCORES, EPC]
    slot_sizes = ((slot_counts.max(axis=0) + 15) // 16 * 16).astype(np.int64)
    slot_sizes = np.maximum(slot_sizes, 16)
    assert slot_sizes.max() <= 128, f"slot overflow {slot_sizes.max()}"
    slot_off = np.concatenate([[0], np.cumsum(slot_sizes)])
    TW = int(slot_off[-1])

    # allgather row of each pair (original (t, k) layout)
    agrow = np.empty(NTOK * K, dtype=np.int64)
    ranks = np.arange(NTOK * K, dtype=np.int64) - starts[flat_e[order]]
    # expert -> (core, slot) mapping
    e2slotoff = np.zeros(POOL, dtype=np.int64)
    for c in range(NCORES):
        for s in range(EPC):
            e2slotoff[slot_expert[c, s]] = c * TW + slot_off[s]
    agrow[order] = e2slotoff[flat_e[order]] + ranks
    agrow = agrow.reshape(NTOK, K)
    return order, counts, starts, tok_sorted, slot_expert, slot_sizes, slot_off, TW, agrow


def _prepare_inputs(x, selected_indices, pattern_weights, base_patterns, cm_w, cm_b,
                    adj_proj, w2_w):
    bf = ml_dtypes.bfloat16
    x2 = np.ascontiguousarray(x.reshape(NTOK, D), dtype=np.float32)
    idx = np.ascontiguousarray(selected_indices.reshape(NTOK, K)).astype(np.int32)
    pw = np.ascontiguousarray(pattern_weights.reshape(NTOK, K), dtype=np.float32)

    # exact constant folding of the cm_b bias into the base patterns
    bp_eff = base_patterns.astype(np.float32) + cm_b.reshape(POOL, M).astype(np.float32) @ adj_proj.astype(np.float32)
    bp_bf = bp_eff.astype(bf)
    adj_bf = adj_proj.astype(bf)
    w2t_bf = np.ascontiguousarray(w2_w.T).astype(bf)
    x2t_bf = np.ascontiguousarray(x2.T).astype(bf)  # [D, NTOK]

    (order, counts, starts, tok_sorted, slot_expert, slot_sizes, slot_off, TW,
     agrow) = _routing(idx)

    cm3 = cm_w.reshape(POOL, M, D)
    in_maps = []
    for c in range(NCORES):
        xgt = np.zeros((D, TW), dtype=bf)
        cmt = np.empty((D, EPC * M), dtype=bf)
        for s in range(EPC):
            e = int(slot_expert[c, s])
            seg = tok_sorted[starts[e]:starts[e] + counts[e]]
            off = int(slot_off[s])
            xgt[:, off:off + len(seg)] = x2t_bf[:, seg]
            cmt[:, s * M:(s + 1) * M] = cm3[e].T.astype(bf)
        agrow_loc = agrow[c * T:(c + 1) * T]            # [T, K]
        gidx = np.ascontiguousarray(
            agrow_loc.reshape(TT, 128, K).transpose(1, 0, 2).reshape(128, TT * K)
        ).astype(np.int32)
        in_maps.append({
            "xgt": xgt,
            "cmt": np.ascontiguousarray(cmt),
            "bp": bp_bf,
            "adjp": adj_bf,
            "w2t": w2t_bf,
            "idx": np.ascontiguousarray(idx[c * T:(c + 1) * T]),
            "pw": np.ascontiguousarray(pw[c * T:(c + 1) * T]),
            "gidx": gidx,
        })
    return in_maps, slot_sizes


def _run(inputs, trace=False):
    in_maps, slot_sizes = _prepare_inputs(
        inputs["x"], inputs["selected_indices"], inputs["pattern_weights"],
        inputs["base_patterns"], inputs["cm_w"], inputs["cm_b"],
        inputs["adj_proj"], inputs["w2_w"])
    nc = _build_program(slot_sizes)
    res = run_bass_kernel_spmd(nc, in_maps, core_ids=list(range(NCORES)), trace=trace)
    out = np.concatenate([res.results[c]["out"] for c in range(NCORES)], axis=0)
    out = out + np.asarray(inputs["w2_b"], dtype=np.float32)[None, :]
    return out.reshape(B, S, D).astype(np.float32), res


def kernel(**inputs) -> np.ndarray:
    out, _ = _run(inputs, trace=False)
    return out


# revision 13
# speedup vs baseline: 1164.0809x; 1.0020x over previous
"""ContextualNeuronPool Trainium2 kernel (8-core SPMD).

Math (per token t, with K=8 selected pool entries p_k = idx[t,k], w = softmax(pattern_weights[t])):
    combined[t, f] = sum_k w_k * bp_eff[p_k, f]                  (base term, via routing matrix A)
                   + (sum_k w_k * (G[p_k] @ x[t])) @ adj_proj    (modulation term, via MoE grouping)
    out[t] = gelu(combined[t]) @ W2^T + w2_b
where G[p] = cm_w block [64, 1024] for pool entry p and bp_eff folds the cm_b bias:
    bp_eff = base_patterns + cm_b.reshape(P, M) @ adj_proj       (exact constant folding, host side)

Sharding:
  Phase A (expert-sharded): core c owns pool entries [64c, 64c+64). Host groups (token, k)
  pairs by pool entry (integer routing logistics only). Per core, entries are sorted by
  token count into 64 slots; slot s is padded to a global slot_sizes[s] (max over cores,
  16-aligned) so all cores run an identical program. Host ships the gathered x columns
  (bf16); each slot's modulation vectors come from one [128d x m] x [128d x 64] matmul
  chain. Pair vectors go to a DRAM table, AllGather'd across the 8 cores.
  Phase B (token-sharded): core c owns tokens [512c, 512c+512). Softmax, routing matrix A,
  base-term matmul A @ bp_eff (overlaps the AllGather), + modulation @ adj_proj, gelu,
  @ W2^T -- all dense matmuls.
"""

import numpy as np
import ml_dtypes

import concourse.bacc as bacc
import concourse.bass as bass
import concourse.tile as tile
import concourse.mybir as mybir
from concourse.bass_utils import run_bass_kernel_spmd
from concourse.masks import make_identity

BF16 = mybir.dt.bfloat16
F32 = mybir.dt.float32
I32 = mybir.dt.int32
AF = mybir.ActivationFunctionType
ALU = mybir.AluOpType

POOL, D, DFF, M = 512, 1024, 4096, 64
B, S, K = 2, 2048, 8
NCORES = 8
NTOK = B * S                  # 4096 tokens
T = NTOK // NCORES            # 512 tokens per core
EPC = POOL // NCORES          # 64 experts (pool entries) per core
DC = D // 128                 # 8 contraction chunks
TT = T // 128                 # 4 token tiles per core
PC = POOL // 128              # 4 pool chunks
FT = DFF // 128               # 32 d_ff tiles
GRP = 16                      # expert slots per DMA load group


def _build_program(slot_sizes):
    slot_off = np.concatenate([[0], np.cumsum(slot_sizes)]).astype(int)
    TW = int(slot_off[-1])          # total packed pair-table width
    NAG = NCORES * TW

    nc = bacc.Bacc("TRN2", target_bir_lowering=False, debug=False, num_devices=NCORES)

    xgt_d = nc.dram_tensor("xgt", [D, TW], BF16, kind="ExternalInput")
    cmt_d = nc.dram_tensor("cmt", [D, EPC * M], BF16, kind="ExternalInput")
    bp_d = nc.dram_tensor("bp", [POOL, DFF], BF16, kind="ExternalInput")
    adj_d = nc.dram_tensor("adjp", [M, DFF], BF16, kind="ExternalInput")
    w2t_d = nc.dram_tensor("w2t", [DFF, D], BF16, kind="ExternalInput")
    idx_d = nc.dram_tensor("idx", [T, K], I32, kind="ExternalInput")
    pw_d = nc.dram_tensor("pw", [T, K], F32, kind="ExternalInput")
    gidx_d = nc.dram_tensor("gidx", [128, TT * K], I32, kind="ExternalInput")
    out_d = nc.dram_tensor("out", [T, D], F32, kind="ExternalOutput")

    with tile.TileContext(nc) as tc:
        with tc.tile_pool(name="const", bufs=1) as const, \
             tc.tile_pool(name="xg", bufs=2) as xg_pool, \
             tc.tile_pool(name="cm", bufs=2) as cm_pool, \
             tc.tile_pool(name="pra", bufs=4) as pr_pool, \
             tc.tile_pool(name="small", bufs=1) as small, \
             tc.tile_pool(name="abuild", bufs=2) as ab_pool, \
             tc.tile_pool(name="rg", bufs=8) as rg_pool, \
             tc.tile_pool(name="rw", bufs=2) as rw_pool, \
             tc.tile_pool(name="w2s", bufs=2) as w2_pool, \
             tc.tile_pool(name="outp", bufs=2) as out_pool, \
             tc.tile_pool(name="dram", bufs=1, space="DRAM") as dram:

            # ---------------- constants / small inputs ----------------
            ident = const.tile([128, 128], BF16)
            make_identity(nc, ident[:])
            iota_f = const.tile([128, POOL], F32)
            nc.gpsimd.iota(iota_f[:], pattern=[[1, POOL]], base=0, channel_multiplier=0,
                           allow_small_or_imprecise_dtypes=True)

            idxf = small.tile([128, TT, K], F32)
            w_sb = small.tile([128, TT, K], F32)
            gidx_sb = small.tile([128, TT * K], I32)
            nc.sync.dma_start(out=gidx_sb[:], in_=gidx_d[:, :])

            idx_i = small.tile([128, TT, K], I32)
            pw_sb = small.tile([128, TT, K], F32)
            for ti in range(TT):
                nc.sync.dma_start(out=idx_i[:, ti], in_=idx_d[ti * 128:(ti + 1) * 128, :])
                nc.sync.dma_start(out=pw_sb[:, ti], in_=pw_d[ti * 128:(ti + 1) * 128, :])

            # softmax over k (per token) + int->float cast of indices
            negmax = small.tile([128, TT, 1], F32)
            sume = small.tile([128, TT, 1], F32)
            rec = small.tile([128, TT, 1], F32)
            for ti in range(TT):
                nc.vector.reduce_max(out=negmax[:, ti], in_=pw_sb[:, ti],
                                     axis=mybir.AxisListType.X, negate=True)
                nc.scalar.activation(out=w_sb[:, ti], in_=pw_sb[:, ti], func=AF.Exp,
                                     bias=negmax[:, ti], scale=1.0, accum_out=sume[:, ti])
                nc.vector.reciprocal(out=rec[:, ti], in_=sume[:, ti])
                nc.vector.tensor_scalar_mul(out=w_sb[:, ti], in0=w_sb[:, ti], scalar1=rec[:, ti])
                nc.vector.tensor_copy(out=idxf[:, ti], in_=idx_i[:, ti])

            pair_tab = dram.tile([TW, M], BF16)
            ag_tab = dram.tile([NAG, M], BF16)

            at_tiles = []  # A^T chunk tiles [128 pool, T] bf16
            for pj in range(PC):
                at_tiles.append(const.tile([128, T], BF16, tag=f"at{pj}", name=f"at{pj}"))
            wmodT = const.tile([M, T], BF16, tag="wmodT")
            stage_tiles = []  # staged base term combined^T tiles [128 f, T] bf16
            act_tiles = []
            for ft in range(FT):
                stage_tiles.append(const.tile([128, T], BF16, tag=f"stg{ft}", name=f"stg{ft}"))
                act_tiles.append(const.tile([128, T], BF16, tag=f"act{ft}", name=f"act{ft}"))

            with tc.tile_pool(name="psA", bufs=3, space="PSUM") as psA, \
                 tc.tile_pool(name="psT", bufs=2, space="PSUM") as psT, \
                 tc.tile_pool(name="psB", bufs=2, space="PSUM") as psB, \
                 tc.tile_pool(name="bpp", bufs=1) as bp_pool:
                bp_tiles = []
                for pj in range(PC):
                    t_ = bp_pool.tile([128, DFF], BF16, tag=f"bp{pj}", name=f"bp{pj}")
                    nc.gpsimd.dma_start(out=t_[:], in_=bp_d[pj * 128:(pj + 1) * 128, :])
                    bp_tiles.append(t_)
                half_off = int(slot_off[EPC // 2])
                # ------------ phase A: per-slot modulation vectors ------------
                for g in range(EPC // GRP):
                    glo, ghi = slot_off[g * GRP], slot_off[(g + 1) * GRP]
                    gw = int(ghi - glo)
                    xg = [xg_pool.tile([128, gw], BF16, tag=f"xgc{j}", name=f"xg{g}_{j}", bufs=2)
                          for j in range(DC)]
                    cm = [cm_pool.tile([128, GRP * M], BF16, tag=f"cmc{j}", name=f"cm{g}_{j}", bufs=2)
                          for j in range(DC)]
                    for j in range(DC):
                        nc.sync.dma_start(out=xg[j][:], in_=xgt_d[j * 128:(j + 1) * 128, glo:ghi])
                        nc.sync.dma_start(
                            out=cm[j][:], in_=cmt_d[j * 128:(j + 1) * 128,
                                                    g * GRP * M:(g + 1) * GRP * M])
                    for s in range(GRP):
                        sl = g * GRP + s
                        m_s = int(slot_sizes[sl])
                        lo = int(slot_off[sl] - glo)
                        ps = psA.tile([128, M], F32)
                        for j in range(DC):
                            nc.tensor.matmul(ps[:m_s, :],
                                             lhsT=xg[j][:, lo:lo + m_s],
                                             rhs=cm[j][:, s * M:(s + 1) * M],
                                             start=(j == 0), stop=(j == DC - 1))
                        pr = pr_pool.tile([128, M], BF16)
                        nc.vector.tensor_copy(out=pr[:m_s, :], in_=ps[:m_s, :])
                        nc.gpsimd.dma_start(
                            out=pair_tab[int(slot_off[sl]):int(slot_off[sl]) + m_s, :],
                            in_=pr[:m_s, :])

                    if g == (EPC // GRP) // 2 - 1:
                        nc.gpsimd.collective_compute(
                            "AllGather", ALU.bypass,
                            replica_groups=[list(range(NCORES))],
                            ins=[pair_tab[0:half_off].opt()],
                            outs=[ag_tab[0:NCORES * half_off].opt()],
                        )

                nc.gpsimd.collective_compute(
                    "AllGather", ALU.bypass,
                    replica_groups=[list(range(NCORES))],
                    ins=[pair_tab[half_off:TW].opt()],
                    outs=[ag_tab[NCORES * half_off:NCORES * TW].opt()],
                )

                # gather pair vectors (issued immediately after the collective)
                rg_tiles = {}
                for ti in range(TT):
                    for k in range(K):
                        rgt = rg_pool.tile([128, M], BF16, tag="rg", name=f"rg{ti}_{k}")
                        nc.gpsimd.indirect_dma_start(
                            out=rgt[:], out_offset=None,
                            in_=ag_tab[:],
                            in_offset=bass.IndirectOffsetOnAxis(
                                ap=gidx_sb[:, ti * K + k: ti * K + k + 1], axis=0),
                        )
                        rg_tiles[(ti, k)] = rgt

                # ------------ routing matrix A (token-major) + transpose ------------
                for ti in range(TT):
                    a_t = ab_pool.tile([128, POOL], BF16)
                    for k in range(K):
                        if k == 0:
                            nc.vector.tensor_scalar(out=a_t[:], in0=iota_f[:],
                                                    scalar1=idxf[:, ti, k:k + 1],
                                                    scalar2=w_sb[:, ti, k:k + 1],
                                                    op0=ALU.is_equal, op1=ALU.mult)
                        else:
                            tmp = ab_pool.tile([128, POOL], BF16, tag="atmp")
                            nc.vector.tensor_scalar(out=tmp[:], in0=iota_f[:],
                                                    scalar1=idxf[:, ti, k:k + 1],
                                                    scalar2=w_sb[:, ti, k:k + 1],
                                                    op0=ALU.is_equal, op1=ALU.mult)
                            nc.vector.tensor_tensor(out=a_t[:], in0=a_t[:], in1=tmp[:], op=ALU.add)
                    for pj in range(PC):
                        pst = psT.tile([128, 128], BF16)
                        nc.tensor.transpose(pst[:], a_t[:, pj * 128:(pj + 1) * 128], ident[:])
                        nc.vector.tensor_copy(out=at_tiles[pj][:, ti * 128:(ti + 1) * 128],
                                              in_=pst[:])

                # ------ pass 1: base term combined^T = A @ bp_eff (overlaps AllGather) ------
                for ft in range(FT):
                    psb = psB.tile([128, T], F32)
                    for pj in range(PC):
                        nc.tensor.matmul(psb[:], lhsT=bp_tiles[pj][:, ft * 128:(ft + 1) * 128],
                                         rhs=at_tiles[pj][:], start=(pj == 0), stop=(pj == PC - 1))
                    nc.vector.tensor_copy(out=stage_tiles[ft][:], in_=psb[:])

                # ------------ weighted k-sum of gathered pair vectors -> wmodT ------------
                for ti in range(TT):
                    rw = rw_pool.tile([128, K, M], F32, tag="rw")
                    for k in range(K):
                        nc.vector.tensor_scalar_mul(out=rw[:, k], in0=rg_tiles[(ti, k)][:],
                                                    scalar1=w_sb[:, ti, k:k + 1])
                    wmod = rw_pool.tile([128, M], F32, tag="wmod")
                    nc.vector.reduce_sum(out=wmod[:], in_=rw[:].rearrange("p k m -> p m k"),
                                         axis=mybir.AxisListType.X)
                    wmod_bf = rw_pool.tile([128, M], BF16, tag="wmodbf")
                    nc.vector.tensor_copy(out=wmod_bf[:], in_=wmod[:])
                    psw = psB.tile([M, 128], BF16, tag="psw", bufs=1)
                    nc.tensor.transpose(psw[:], wmod_bf[:], ident[:])
                    nc.vector.tensor_copy(out=wmodT[:, ti * 128:(ti + 1) * 128], in_=psw[:])

            adj_sb = const.tile([M, DFF], BF16, tag="adj")
            nc.sync.dma_start(out=adj_sb[:], in_=adj_d[:, :])

            # ------- pass 2 (adj term + gelu) fused with first half of W2 matmul -------
            def b2_half(psO, trange, reload_tag):
                for fc in range(FT):
                    w2c = w2_pool.tile([128, D], BF16, tag="w2c", name=f"w2c{reload_tag}")
                    nc.sync.dma_start(out=w2c[:], in_=w2t_d[fc * 128:(fc + 1) * 128, :])
                    for t in trange:
                        for dd in range(2):
                            nc.tensor.matmul(psO[(t, dd)][:],
                                             lhsT=act_tiles[fc][:, t * 128:(t + 1) * 128],
                                             rhs=w2c[:, dd * 512:(dd + 1) * 512],
                                             start=(fc == 0), stop=(fc == FT - 1))

            def drain_half(psO, trange):
                for t in trange:
                    for dd in range(2):
                        ob = out_pool.tile([128, 512], F32)
                        nc.vector.tensor_copy(out=ob[:], in_=psO[(t, dd)][:])
                        nc.sync.dma_start(
                            out=out_d[t * 128:(t + 1) * 128, dd * 512:(dd + 1) * 512],
                            in_=ob[:])

            def pass2_half(psC, h):
                sl = slice(h * 256, (h + 1) * 256)
                for ft in range(FT):
                    psc = psC.tile([128, 256], F32, tag="psc", name=f"psc{h}_{ft}")
                    nc.tensor.matmul(psc[:], lhsT=adj_sb[:, ft * 128:(ft + 1) * 128],
                                     rhs=wmodT[:, sl], start=True, stop=True)
                    comb = ab_pool.tile([128, 256], BF16, tag="atmp", name=f"comb{h}_{ft}")
                    nc.vector.tensor_tensor(out=comb[:], in0=stage_tiles[ft][:, sl],
                                            in1=psc[:], op=ALU.add)
                    nc.scalar.activation(out=act_tiles[ft][:, sl], in_=comb[:], func=AF.Gelu)

            with tc.tile_pool(name="psOa", bufs=1, space="PSUM") as psOa_pool:
                psOa = {}
                for t in (0, 1):
                    for dd in range(2):
                        psOa[(t, dd)] = psOa_pool.tile([128, 512], F32, tag=f"oa{t}_{dd}",
                                                       name=f"opsa{t}_{dd}")
                with tc.tile_pool(name="psC", bufs=3, space="PSUM") as psC:
                    pass2_half(psC, 0)
                    b2_half(psOa, (0, 1), "w2a")
                    pass2_half(psC, 1)
                with tc.tile_pool(name="psOb", bufs=1, space="PSUM") as psOb_pool:
                    psOb = {}
                    for t in (2, 3):
                        for dd in range(2):
                            psOb[(t, dd)] = psOb_pool.tile([128, 512], F32, tag=f"ob{t}_{dd}",
                                                           name=f"opsb{t}_{dd}")
                    b2_half(psOb, (2, 3), "w2b")
                    drain_half(psOa, (0, 1))
                    drain_half(psOb, (2, 3))

    nc.compile()
    return nc


def _routing(idx):
    """Group (t, k) pairs by pool entry; build per-core slot packing (sorted by count)."""
    flat_e = idx.ravel()
    order = np.argsort(flat_e, kind="stable")  # pairs sorted by (expert, t, k)
    counts = np.bincount(flat_e, minlength=POOL)
    starts = np.zeros(POOL, dtype=np.int64)
    starts[1:] = np.cumsum(counts)[:-1]
    tok_sorted = (np.arange(NTOK * K, dtype=np.int64) // K)[order]

    # per core: experts sorted by count desc -> slots
    slot_expert = np.zeros((NCORES, EPC), dtype=np.int64)
    for c in range(NCORES):
        cnt = counts[c * EPC:(c + 1) * EPC]
        slot_expert[c] = c * EPC + np.argsort(-cnt, kind="stable")
    slot_counts = counts[slot_expert]                       # [NCORES, EPC]
    slot_sizes = ((slot_counts.max(axis=0) + 15) // 16 * 16).astype(np.int64)
    slot_sizes = np.maximum(slot_sizes, 16)
    assert slot_sizes.max() <= 128, f"slot overflow {slot_sizes.max()}"
    slot_off = np.concatenate([[0], np.cumsum(slot_sizes)])
    TW = int(slot_off[-1])

    # allgather row of each pair (original (t, k) layout)
    agrow = np.empty(NTOK * K, dtype=np.int64)
    ranks = np.arange(NTOK * K, dtype=np.int64) - starts[flat_e[order]]
    # expert -> allgather-table row base. The table holds two contiguous
    # AllGather blocks: slots [0, EPC/2) rank-major, then slots [EPC/2, EPC).
    H = int(slot_off[EPC // 2])
    e2slotoff = np.zeros(POOL, dtype=np.int64)
    for c in range(NCORES):
        for s in range(EPC):
            so = int(slot_off[s])
            if s < EPC // 2:
                base = c * H + so
            else:
                base = NCORES * H + c * (TW - H) + (so - H)
            e2slotoff[slot_expert[c, s]] = base
    agrow[order] = e2slotoff[flat_e[order]] + ranks
    agrow = agrow.reshape(NTOK, K)
    return order, counts, starts, tok_sorted, slot_expert, slot_sizes, slot_off, TW, agrow


def _prepare_inputs(x, selected_indices, pattern_weights, base_patterns, cm_w, cm_b,
                    adj_proj, w2_w):
    bf = ml_dtypes.bfloat16
    x2 = np.ascontiguousarray(x.reshape(NTOK, D), dtype=np.float32)
    idx = np.ascontiguousarray(selected_indices.reshape(NTOK, K)).astype(np.int32)
    pw = np.ascontiguousarray(pattern_weights.reshape(NTOK, K), dtype=np.float32)

    # exact constant folding of the cm_b bias into the base patterns
    bp_eff = base_patterns.astype(np.float32) + cm_b.reshape(POOL, M).astype(np.float32) @ adj_proj.astype(np.float32)
    bp_bf = bp_eff.astype(bf)
    adj_bf = adj_proj.astype(bf)
    w2t_bf = np.ascontiguousarray(w2_w.T).astype(bf)
    x2t_bf = np.ascontiguousarray(x2.T).astype(bf)  # [D, NTOK]

    (order, counts, starts, tok_sorted, slot_expert, slot_sizes, slot_off, TW,
     agrow) = _routing(idx)

    cm3 = cm_w.reshape(POOL, M, D)
    in_maps = []
    for c in range(NCORES):
        xgt = np.zeros((D, TW), dtype=bf)
        cmt = np.empty((D, EPC * M), dtype=bf)
        for s in range(EPC):
            e = int(slot_expert[c, s])
            seg = tok_sorted[starts[e]:starts[e] + counts[e]]
            off = int(slot_off[s])
            xgt[:, off:off + len(seg)] = x2t_bf[:, seg]
            cmt[:, s * M:(s + 1) * M] = cm3[e].T.astype(bf)
        agrow_loc = agrow[c * T:(c + 1) * T]            # [T, K]
        gidx = np.ascontiguousarray(
            agrow_loc.reshape(TT, 128, K).transpose(1, 0, 2).reshape(128, TT * K)
        ).astype(np.int32)
        in_maps.append({
            "xgt": xgt,
            "cmt": np.ascontiguousarray(cmt),
            "bp": bp_bf,
            "adjp": adj_bf,
            "w2t": w2t_bf,
            "idx": np.ascontiguousarray(idx[c * T:(c + 1) * T]),
            "pw": np.ascontiguousarray(pw[c * T:(c + 1) * T]),
            "gidx": gidx,
        })
    return in_maps, slot_sizes


def _run(inputs, trace=False):
    in_maps, slot_sizes = _prepare_inputs(
        inputs["x"], inputs["selected_indices"], inputs["pattern_weights"],
        inputs["base_patterns"], inputs["cm_w"], inputs["cm_b"],
        inputs["adj_proj"], inputs["w2_w"])
    nc = _build_program(slot_sizes)
    res = run_bass_kernel_spmd(nc, in_maps, core_ids=list(range(NCORES)), trace=trace)
    out = np.concatenate([res.results[c]["out"] for c in range(NCORES)], axis=0)
    out = out + np.asarray(inputs["w2_b"], dtype=np.float32)[None, :]
    return out.reshape(B, S, D).astype(np.float32), res


def kernel(**inputs) -> np.ndarray:
    out, _ = _run(inputs, trace=False)
    return out


# revision 18
# speedup vs baseline: 1172.1021x; 1.0069x over previous
"""ContextualNeuronPool Trainium2 kernel (8-core SPMD).

Math (per token t, with K=8 selected pool entries p_k = idx[t,k], w = softmax(pattern_weights[t])):
    combined[t, f] = sum_k w_k * bp_eff[p_k, f]                  (base term, via routing matrix A)
                   + (sum_k w_k * (G[p_k] @ x[t])) @ adj_proj    (modulation term, via MoE grouping)
    out[t] = gelu(combined[t]) @ W2^T + w2_b
where G[p] = cm_w block [64, 1024] for pool entry p and bp_eff folds the cm_b bias:
    bp_eff = base_patterns + cm_b.reshape(P, M) @ adj_proj       (exact constant folding, host side)

Sharding:
  Phase A (expert-sharded): core c owns pool entries [64c, 64c+64). Host groups (token, k)
  pairs by pool entry (integer routing logistics only). Per core, entries are sorted by
  token count into 64 slots; slot s is padded to a global slot_sizes[s] (max over cores,
  16-aligned) so all cores run an identical program. Host ships the gathered x columns
  (bf16); each slot's modulation vectors come from one [128d x m] x [128d x 64] matmul
  chain. Pair vectors go to a DRAM table, AllGather'd across the 8 cores, then each core
  gathers its tokens' pair rows by index (indirect DMA).
  Phase B (token-sharded): core c owns tokens [512c, 512c+512). Softmax, routing matrix A,
  base-term matmul A @ bp_eff (overlaps the AllGather), + modulation @ adj_proj, gelu,
  @ W2^T -- all dense matmuls, pipelined by token halves.
"""

import numpy as np
import ml_dtypes

import concourse.bacc as bacc
import concourse.bass as bass
import concourse.tile as tile
import concourse.mybir as mybir
from concourse.bass_utils import run_bass_kernel_spmd
from concourse.masks import make_identity

BF16 = mybir.dt.bfloat16
F32 = mybir.dt.float32
I32 = mybir.dt.int32
AF = mybir.ActivationFunctionType
ALU = mybir.AluOpType

POOL, D, DFF, M = 512, 1024, 4096, 64
B, S, K = 2, 2048, 8
NCORES = 8
NTOK = B * S                  # 4096 tokens
T = NTOK // NCORES            # 512 tokens per core
EPC = POOL // NCORES          # 64 experts (pool entries) per core
DC = D // 128                 # 8 contraction chunks
TT = T // 128                 # 4 token tiles per core
PC = POOL // 128              # 4 pool chunks
FT = DFF // 128               # 32 d_ff tiles
GRP = 16                      # expert slots per DMA load group


def _build_program(slot_sizes):
    slot_off = np.concatenate([[0], np.cumsum(slot_sizes)]).astype(int)
    TW = int(slot_off[-1])          # total packed pair-table width
    NAG = NCORES * TW

    nc = bacc.Bacc("TRN2", target_bir_lowering=False, debug=False, num_devices=NCORES)

    xgt_d = nc.dram_tensor("xgt", [D, TW], BF16, kind="ExternalInput")
    cmt_d = nc.dram_tensor("cmt", [D, EPC * M], BF16, kind="ExternalInput")
    bp_d = nc.dram_tensor("bp", [POOL, DFF], BF16, kind="ExternalInput")
    adj_d = nc.dram_tensor("adjp", [M, DFF], BF16, kind="ExternalInput")
    w2t_d = nc.dram_tensor("w2t", [DFF, D], BF16, kind="ExternalInput")
    idx_d = nc.dram_tensor("idx", [T, K], I32, kind="ExternalInput")
    pw_d = nc.dram_tensor("pw", [T, K], F32, kind="ExternalInput")
    gidx_d = nc.dram_tensor("gidx", [128, TT * K], I32, kind="ExternalInput")
    out_d = nc.dram_tensor("out", [T, D], F32, kind="ExternalOutput")

    with tile.TileContext(nc) as tc:
        with tc.tile_pool(name="const", bufs=1) as const, \
             tc.tile_pool(name="xg", bufs=2) as xg_pool, \
             tc.tile_pool(name="cm", bufs=2) as cm_pool, \
             tc.tile_pool(name="pra", bufs=4) as pr_pool, \
             tc.tile_pool(name="small", bufs=1) as small, \
             tc.tile_pool(name="abuild", bufs=2) as ab_pool, \
             tc.tile_pool(name="rg", bufs=8) as rg_pool, \
             tc.tile_pool(name="rw", bufs=2) as rw_pool, \
             tc.tile_pool(name="w2s", bufs=2) as w2_pool, \
             tc.tile_pool(name="outp", bufs=2) as out_pool, \
             tc.tile_pool(name="dram", bufs=1, space="DRAM") as dram:

            # ---------------- constants / small inputs ----------------
            ident = const.tile([128, 128], BF16)
            make_identity(nc, ident[:])
            iota_f = const.tile([128, POOL], F32)
            nc.gpsimd.iota(iota_f[:], pattern=[[1, POOL]], base=0, channel_multiplier=0,
                           allow_small_or_imprecise_dtypes=True)

            idxf = small.tile([128, TT, K], F32)
            w_sb = small.tile([128, TT, K], F32)
            gidx_sb = small.tile([128, TT * K], I32)
            nc.sync.dma_start(out=gidx_sb[:], in_=gidx_d[:, :])

            idx_i = small.tile([128, TT, K], I32)
            pw_sb = small.tile([128, TT, K], F32)
            for ti in range(TT):
                nc.sync.dma_start(out=idx_i[:, ti], in_=idx_d[ti * 128:(ti + 1) * 128, :])
                nc.sync.dma_start(out=pw_sb[:, ti], in_=pw_d[ti * 128:(ti + 1) * 128, :])

            # softmax over k (per token) + int->float cast of indices
            negmax = small.tile([128, TT, 1], F32)
            sume = small.tile([128, TT, 1], F32)
            rec = small.tile([128, TT, 1], F32)
            for ti in range(TT):
                nc.vector.reduce_max(out=negmax[:, ti], in_=pw_sb[:, ti],
                                     axis=mybir.AxisListType.X, negate=True)
                nc.scalar.activation(out=w_sb[:, ti], in_=pw_sb[:, ti], func=AF.Exp,
                                     bias=negmax[:, ti], scale=1.0, accum_out=sume[:, ti])
                nc.vector.reciprocal(out=rec[:, ti], in_=sume[:, ti])
                nc.vector.tensor_scalar_mul(out=w_sb[:, ti], in0=w_sb[:, ti], scalar1=rec[:, ti])
                nc.vector.tensor_copy(out=idxf[:, ti], in_=idx_i[:, ti])

            pair_tab = dram.tile([TW, M], BF16)
            ag_tab = dram.tile([NAG, M], BF16, addr_space="Shared")

            at_tiles = []  # A^T chunk tiles [128 pool, T] bf16
            for pj in range(PC):
                at_tiles.append(const.tile([128, T], BF16, tag=f"at{pj}", name=f"at{pj}"))
            wmodT = const.tile([M, T], BF16, tag="wmodT")
            stage_tiles = []  # staged base term combined^T tiles [128 f, T] bf16
            act_tiles = []
            for ft in range(FT):
                stage_tiles.append(const.tile([128, T], BF16, tag=f"stg{ft}", name=f"stg{ft}"))
                act_tiles.append(const.tile([128, T], BF16, tag=f"act{ft}", name=f"act{ft}"))

            with tc.tile_pool(name="psA", bufs=3, space="PSUM") as psA, \
                 tc.tile_pool(name="psT", bufs=2, space="PSUM") as psT, \
                 tc.tile_pool(name="psB", bufs=2, space="PSUM") as psB, \
                 tc.tile_pool(name="bpp", bufs=1) as bp_pool:
                bp_tiles = []
                for pj in range(PC):
                    t_ = bp_pool.tile([128, DFF], BF16, tag=f"bp{pj}", name=f"bp{pj}")
                    nc.gpsimd.dma_start(out=t_[:], in_=bp_d[pj * 128:(pj + 1) * 128, :])
                    bp_tiles.append(t_)
                # ------------ phase A: per-slot modulation vectors ------------
                for g in range(EPC // GRP):
                    glo, ghi = slot_off[g * GRP], slot_off[(g + 1) * GRP]
                    gw = int(ghi - glo)
                    xg = [xg_pool.tile([128, gw], BF16, tag=f"xgc{j}", name=f"xg{g}_{j}", bufs=2)
                          for j in range(DC)]
                    cm = [cm_pool.tile([128, GRP * M], BF16, tag=f"cmc{j}", name=f"cm{g}_{j}", bufs=2)
                          for j in range(DC)]
                    for j in range(DC):
                        nc.sync.dma_start(out=xg[j][:], in_=xgt_d[j * 128:(j + 1) * 128, glo:ghi])
                        nc.sync.dma_start(
                            out=cm[j][:], in_=cmt_d[j * 128:(j + 1) * 128,
                                                    g * GRP * M:(g + 1) * GRP * M])
                    for s in range(GRP):
                        sl = g * GRP + s
                        m_s = int(slot_sizes[sl])
                        lo = int(slot_off[sl] - glo)
                        ps = psA.tile([128, M], F32)
                        for j in range(DC):
                            nc.tensor.matmul(ps[:m_s, :],
                                             lhsT=xg[j][:, lo:lo + m_s],
                                             rhs=cm[j][:, s * M:(s + 1) * M],
                                             start=(j == 0), stop=(j == DC - 1))
                        pr = pr_pool.tile([128, M], BF16)
                        nc.vector.tensor_copy(out=pr[:m_s, :], in_=ps[:m_s, :])
                        nc.gpsimd.dma_start(
                            out=pair_tab[int(slot_off[sl]):int(slot_off[sl]) + m_s, :],
                            in_=pr[:m_s, :])

                nc.gpsimd.collective_compute(
                    "AllGather", ALU.bypass,
                    replica_groups=[list(range(NCORES))],
                    ins=[pair_tab[:].opt()], outs=[ag_tab[:].opt()],
                )

                # gather pair vectors (issued immediately after the collective)
                rg_tiles = {}
                for ti in range(TT):
                    for k in range(K):
                        rgt = rg_pool.tile([128, M], BF16, tag="rg", name=f"rg{ti}_{k}")
                        nc.gpsimd.indirect_dma_start(
                            out=rgt[:], out_offset=None,
                            in_=ag_tab[:],
                            in_offset=bass.IndirectOffsetOnAxis(
                                ap=gidx_sb[:, ti * K + k: ti * K + k + 1], axis=0),
                        )
                        rg_tiles[(ti, k)] = rgt

                # ------------ routing matrix A (token-major) + transpose ------------
                for ti in range(TT):
                    a_t = ab_pool.tile([128, POOL], BF16)
                    for k in range(K):
                        if k == 0:
                            nc.vector.tensor_scalar(out=a_t[:], in0=iota_f[:],
                                                    scalar1=idxf[:, ti, k:k + 1],
                                                    scalar2=w_sb[:, ti, k:k + 1],
                                                    op0=ALU.is_equal, op1=ALU.mult)
                        else:
                            tmp = ab_pool.tile([128, POOL], BF16, tag="atmp")
                            nc.vector.tensor_scalar(out=tmp[:], in0=iota_f[:],
                                                    scalar1=idxf[:, ti, k:k + 1],
                                                    scalar2=w_sb[:, ti, k:k + 1],
                                                    op0=ALU.is_equal, op1=ALU.mult)
                            nc.vector.tensor_tensor(out=a_t[:], in0=a_t[:], in1=tmp[:], op=ALU.add)
                    for pj in range(PC):
                        pst = psT.tile([128, 128], BF16)
                        nc.tensor.transpose(pst[:], a_t[:, pj * 128:(pj + 1) * 128], ident[:])
                        nc.vector.tensor_copy(out=at_tiles[pj][:, ti * 128:(ti + 1) * 128],
                                              in_=pst[:])

                # ------ pass 1: base term combined^T = A @ bp_eff (overlaps AllGather) ------
                for ft in range(FT):
                    psb = psB.tile([128, T], F32)
                    for pj in range(PC):
                        nc.tensor.matmul(psb[:], lhsT=bp_tiles[pj][:, ft * 128:(ft + 1) * 128],
                                         rhs=at_tiles[pj][:], start=(pj == 0), stop=(pj == PC - 1))
                    nc.vector.tensor_copy(out=stage_tiles[ft][:], in_=psb[:])

                # ------------ weighted k-sum of gathered pair vectors -> wmodT ------------
                for ti in range(TT):
                    rw = rw_pool.tile([128, K, M], F32, tag="rw")
                    for k in range(K):
                        nc.vector.tensor_scalar_mul(out=rw[:, k], in0=rg_tiles[(ti, k)][:],
                                                    scalar1=w_sb[:, ti, k:k + 1])
                    wmod = rw_pool.tile([128, M], F32, tag="wmod")
                    nc.vector.reduce_sum(out=wmod[:], in_=rw[:].rearrange("p k m -> p m k"),
                                         axis=mybir.AxisListType.X)
                    wmod_bf = rw_pool.tile([128, M], BF16, tag="wmodbf")
                    nc.vector.tensor_copy(out=wmod_bf[:], in_=wmod[:])
                    psw = psB.tile([M, 128], BF16, tag="psw", bufs=1)
                    nc.tensor.transpose(psw[:], wmod_bf[:], ident[:])
                    nc.vector.tensor_copy(out=wmodT[:, ti * 128:(ti + 1) * 128], in_=psw[:])

            adj_sb = const.tile([M, DFF], BF16, tag="adj")
            nc.sync.dma_start(out=adj_sb[:], in_=adj_d[:, :])

            # ------- pass 2 (adj term + gelu) fused with first half of W2 matmul -------
            def b2_half(psO, trange, reload_tag):
                for fc in range(FT):
                    w2c = w2_pool.tile([128, D], BF16, tag="w2c", name=f"w2c{reload_tag}")
                    nc.sync.dma_start(out=w2c[:], in_=w2t_d[fc * 128:(fc + 1) * 128, :])
                    for t in trange:
                        for dd in range(2):
                            nc.tensor.matmul(psO[(t, dd)][:],
                                             lhsT=act_tiles[fc][:, t * 128:(t + 1) * 128],
                                             rhs=w2c[:, dd * 512:(dd + 1) * 512],
                                             start=(fc == 0), stop=(fc == FT - 1))

            def drain_half(psO, trange):
                for t in trange:
                    for dd in range(2):
                        ob = out_pool.tile([128, 512], F32)
                        nc.vector.tensor_copy(out=ob[:], in_=psO[(t, dd)][:])
                        nc.sync.dma_start(
                            out=out_d[t * 128:(t + 1) * 128, dd * 512:(dd + 1) * 512],
                            in_=ob[:])

            def pass2_half(psC, h):
                sl = slice(h * 256, (h + 1) * 256)
                for ft in range(FT):
                    psc = psC.tile([128, 256], F32, tag="psc", name=f"psc{h}_{ft}")
                    nc.tensor.matmul(psc[:], lhsT=adj_sb[:, ft * 128:(ft + 1) * 128],
                                     rhs=wmodT[:, sl], start=True, stop=True)
                    comb = ab_pool.tile([128, 256], BF16, tag="atmp", name=f"comb{h}_{ft}")
                    nc.vector.tensor_tensor(out=comb[:], in0=stage_tiles[ft][:, sl],
                                            in1=psc[:], op=ALU.add)
                    nc.scalar.activation(out=act_tiles[ft][:, sl], in_=comb[:], func=AF.Gelu)

            with tc.tile_pool(name="psOa", bufs=1, space="PSUM") as psOa_pool:
                psOa = {}
                for t in (0, 1):
                    for dd in range(2):
                        psOa[(t, dd)] = psOa_pool.tile([128, 512], F32, tag=f"oa{t}_{dd}",
                                                       name=f"opsa{t}_{dd}")
                with tc.tile_pool(name="psC", bufs=3, space="PSUM") as psC:
                    pass2_half(psC, 0)
                    b2_half(psOa, (0, 1), "w2a")
                    pass2_half(psC, 1)
                with tc.tile_pool(name="psOb", bufs=1, space="PSUM") as psOb_pool:
                    psOb = {}
                    for t in (2, 3):
                        for dd in range(2):
                            psOb[(t, dd)] = psOb_pool.tile([128, 512], F32, tag=f"ob{t}_{dd}",
                                                           name=f"opsb{t}_{dd}")
                    b2_half(psOb, (2, 3), "w2b")
                    drain_half(psOa, (0, 1))
                    drain_half(psOb, (2, 3))

    nc.compile()
    return nc


def _routing(idx):
    """Group (t, k) pairs by pool entry; build per-core slot packing (sorted by count)."""
    flat_e = idx.ravel()
    order = np.argsort(flat_e, kind="stable")  # pairs sorted by (expert, t, k)
    counts = np.bincount(flat_e, minlength=POOL)
    starts = np.zeros(POOL, dtype=np.int64)
    starts[1:] = np.cumsum(counts)[:-1]
    tok_sorted = (np.arange(NTOK * K, dtype=np.int64) // K)[order]

    # per core: experts sorted by count desc -> slots
    slot_expert = np.zeros((NCORES, EPC), dtype=np.int64)
    for c in range(NCORES):
        cnt = counts[c * EPC:(c + 1) * EPC]
        slot_expert[c] = c * EPC + np.argsort(-cnt, kind="stable")
    slot_counts = counts[slot_expert]                       # [NCORES, EPC]
    slot_sizes = ((slot_counts.max(axis=0) + 15) // 16 * 16).astype(np.int64)
    slot_sizes = np.maximum(slot_sizes, 16)
    assert slot_sizes.max() <= 128, f"slot overflow {slot_sizes.max()}"
    slot_off = np.concatenate([[0], np.cumsum(slot_sizes)])
    TW = int(slot_off[-1])

    # allgather row of each pair (original (t, k) layout)
    agrow = np.empty(NTOK * K, dtype=np.int64)
    ranks = np.arange(NTOK * K, dtype=np.int64) - starts[flat_e[order]]
    e2slotoff = np.zeros(POOL, dtype=np.int64)
    for c in range(NCORES):
        for s in range(EPC):
            e2slotoff[slot_expert[c, s]] = c * TW + slot_off[s]
    agrow[order] = e2slotoff[flat_e[order]] + ranks
    agrow = agrow.reshape(NTOK, K)
    return order, counts, starts, tok_sorted, slot_expert, slot_sizes, slot_off, TW, agrow


def _prepare_inputs(x, selected_indices, pattern_weights, base_patterns, cm_w, cm_b,
                    adj_proj, w2_w):
    bf = ml_dtypes.bfloat16
    x2 = np.ascontiguousarray(x.reshape(NTOK, D), dtype=np.float32)
    idx = np.ascontiguousarray(selected_indices.reshape(NTOK, K)).astype(np.int32)
    pw = np.ascontiguousarray(pattern_weights.reshape(NTOK, K), dtype=np.float32)

    # exact constant folding of the cm_b bias into the base patterns
    bp_eff = base_patterns.astype(np.float32) + cm_b.reshape(POOL, M).astype(np.float32) @ adj_proj.astype(np.float32)
    bp_bf = bp_eff.astype(bf)
    adj_bf = adj_proj.astype(bf)
    w2t_bf = np.ascontiguousarray(w2_w.T).astype(bf)
    x2t_bf = np.ascontiguousarray(x2.T).astype(bf)  # [D, NTOK]

    (order, counts, starts, tok_sorted, slot_expert, slot_sizes, slot_off, TW,
     agrow) = _routing(idx)

    cm3 = cm_w.reshape(POOL, M, D)
    in_maps = []
    for c in range(NCORES):
        xgt = np.zeros((D, TW), dtype=bf)
        cmt = np.empty((D, EPC * M), dtype=bf)
        for s in range(EPC):
            e = int(slot_expert[c, s])
            seg = tok_sorted[starts[e]:starts[e] + counts[e]]
            off = int(slot_off[s])
            xgt[:, off:off + len(seg)] = x2t_bf[:, seg]
            cmt[:, s * M:(s + 1) * M] = cm3[e].T.astype(bf)
        agrow_loc = agrow[c * T:(c + 1) * T]            # [T, K]
        gidx = np.ascontiguousarray(
            agrow_loc.reshape(TT, 128, K).transpose(1, 0, 2).reshape(128, TT * K)
        ).astype(np.int32)
        in_maps.append({
            "xgt": xgt,
            "cmt": np.ascontiguousarray(cmt),
            "bp": bp_bf,
            "adjp": adj_bf,
            "w2t": w2t_bf,
            "idx": np.ascontiguousarray(idx[c * T:(c + 1) * T]),
            "pw": np.ascontiguousarray(pw[c * T:(c + 1) * T]),
            "gidx": gidx,
        })
    return in_maps, slot_sizes


def _run(inputs, trace=False):
    in_maps, slot_sizes = _prepare_inputs(
        inputs["x"], inputs["selected_indices"], inputs["pattern_weights"],
        inputs["base_patterns"], inputs["cm_w"], inputs["cm_b"],
        inputs["adj_proj"], inputs["w2_w"])
    nc = _build_program(slot_sizes)
    res = run_bass_kernel_spmd(nc, in_maps, core_ids=list(range(NCORES)), trace=trace)
    out = np.concatenate([res.results[c]["out"] for c in range(NCORES)], axis=0)
    out = out + np.asarray(inputs["w2_b"], dtype=np.float32)[None, :]
    return out.reshape(B, S, D).astype(np.float32), res


def kernel(**inputs) -> np.ndarray:
    out, _ = _run(inputs, trace=False)
    return out
